# revision 1
# baseline (speedup 1.0000x reference)
"""BERT encoder (B=16, S=512, H=768, L=12, F=3072, NH=12) on 8 trn2 NeuronCores.

Sharding: pure data-parallel over batch -- each core processes 2 samples
(1024 tokens). Weights are replicated (cast to bf16 host-side), activations
stay feature-major on-chip: xT[f, t] with f on partitions, so every linear
layer is matmul(out=yT, lhsT=W, rhs=xT) with no transposes.

Per-layer pipeline: LN1 -> V(both samples) -> per sample s: [Q(s), K(s),
attention(s), Wo(s)+residual] -> LN2 -> FFN per half.  The per-sample loop
shares one 8-bank PSUM budget so attention(s0) overlaps Q/K(s1); score
matmuls for the two heads of a pair go to distinct PE row groups
(K=64 each) so they pack into the array.  Softmax runs in the transposed
score layout without max-subtraction (scores are O(1) for this model); the
denominator falls out of the attention matmul via an appended ones-column
on V; per-sample denominator rows are DMA-gathered into one [12,512] tile
for a single batched DVE reciprocal (a [1,512] reciprocal costs the same
4us as [12,512]), then row-broadcast via idle-GpSimd partition_broadcast.
Residual adds read the matmul PSUM directly via scalar_tensor_tensor
(bias + add in one DVE op).  LN stats Ln/Exp ops for both halves are
clustered to avoid exp<->gelu ACT-table ping-pong; LN's x^2 runs on GpSimd
so the LN-entry stats matmuls aren't starved behind residual-add work in
the DVE FIFO.  Residual stream fp32; matmul operands bf16.
"""

import sys

for _p in ("/opt/trn_rl_repo",):
    if _p not in sys.path:
        sys.path.insert(0, _p)

import numpy as np
import ml_dtypes

import concourse.bass as bass
import concourse.tile as tile
from concourse import bacc, mybir
from concourse.bass_utils import run_bass_kernel_spmd
from concourse.masks import make_identity

AF = mybir.ActivationFunctionType
ALU = mybir.AluOpType
F32 = mybir.dt.float32
F32R = mybir.dt.float32r
BF16 = mybir.dt.bfloat16
I32 = mybir.dt.int32

B, S, H, L, FF, V, NH = 16, 512, 768, 12, 3072, 30522, 12
HD = H // NH  # 64
NCORES = 8
BPC = B // NCORES  # samples per core = 2
T = BPC * S  # tokens per core = 1024
HC = H // 128  # feature chunks = 6
FC = FF // 128  # ffn chunks = 24
TC = T // 128  # token chunks = 8
NT = T // 512  # 512-token column tiles = 2
EPS_EMB, EPS_LN = 1e-12, 1e-5
VH = 65  # per-head v columns: 64 v + 1 ones (denominator trick)


def _r32(ap):
    return ap.bitcast(F32R)


class Ctx:
    pass


def build_nc(num_layers=L):
    nc = bacc.Bacc("TRN2", target_bir_lowering=False, debug=False,
                   num_devices=NCORES)

    ids = nc.declare_dram_parameter("ids", [T], I32, isOutput=False)
    word_emb = nc.declare_dram_parameter("word_emb", [V, H], F32, isOutput=False)
    ppt = nc.declare_dram_parameter("ppt", [S, H], F32, isOutput=False)
    ln_e = nc.declare_dram_parameter("ln_e", [2, H], F32, isOutput=False)
    c = Ctx()
    c.lnp = nc.declare_dram_parameter("lnp", [L, 4, H], F32, isOutput=False)
    c.wq = nc.declare_dram_parameter("wq", [L, H, H], BF16, isOutput=False)
    c.wk = nc.declare_dram_parameter("wk", [L, H, H], BF16, isOutput=False)
    c.wv = nc.declare_dram_parameter("wv", [L, H, H], BF16, isOutput=False)
    c.wo = nc.declare_dram_parameter("wo", [L, H, H], BF16, isOutput=False)
    c.w1 = nc.declare_dram_parameter("w1", [L, H, FF], BF16, isOutput=False)
    c.w2 = nc.declare_dram_parameter("w2", [L, FF, H], BF16, isOutput=False)
    c.bqkvo = nc.declare_dram_parameter("bqkvo", [L, 4, H], F32, isOutput=False)
    c.b1 = nc.declare_dram_parameter("b1", [L, FF], F32, isOutput=False)
    c.b2 = nc.declare_dram_parameter("b2", [L, H], F32, isOutput=False)
    xt_out = nc.declare_dram_parameter("xt_out", [H, T], F32, isOutput=True)

    def dram_bcast(ap_1d, parts):
        a = ap_1d
        return bass.AP(tensor=a.tensor, offset=a.offset, ap=[[0, parts], *a.ap])

    c.dram_bcast = dram_bcast

    with tile.TileContext(nc) as tc:
        with (
            tc.tile_pool(name="persist", bufs=1) as pp,
            tc.tile_pool(name="xpool", bufs=1) as xp,
        ):
            identity = pp.tile([128, 128], F32)
            make_identity(nc, identity[:])
            c.ones_col = pp.tile([128, 1], F32)
            nc.vector.memset(c.ones_col[:], 1.0)
            c.ones_col_bf = pp.tile([128, 1], BF16)
            nc.vector.memset(c.ones_col_bf[:], 1.0)
            c.ones_row = pp.tile([1, 128], F32)
            nc.vector.memset(c.ones_row[:], 1.0)
            c.ones_row_bf = pp.tile([1, 128], BF16)
            nc.vector.memset(c.ones_row_bf[:], 1.0)
            eps_e = pp.tile([128, 1], F32)
            nc.vector.memset(eps_e[:], EPS_EMB)
            c.eps_l = pp.tile([1, 1], F32)
            nc.vector.memset(c.eps_l[:], EPS_LN)

            xT = xp.tile([128, HC, T], F32)  # residual stream, feature-major

            # ---------------- embedding ----------------
            with (
                tc.tile_pool(name="emb", bufs=2) as ep,
                tc.tile_pool(name="embc", bufs=1) as ec,
                tc.tile_pool(name="embps", bufs=2, space="PSUM") as ps_e,
            ):
                s_b = ec.tile([128, H], F32)
                nc.sync.dma_start(out=s_b[:], in_=dram_bcast(ln_e[0], 128))
                b_b = ec.tile([128, H], F32)
                nc.sync.dma_start(out=b_b[:], in_=dram_bcast(ln_e[1], 128))
                pptb = ec.tile([128, S // 128, H], F32)
                nc.sync.dma_start(
                    out=pptb[:], in_=ppt[:].rearrange("(c p) h -> p c h", p=128))
                for tch in range(TC):
                    idx = ep.tile([128, 1], I32)
                    nc.sync.dma_start(out=idx[:],
                                      in_=ids[tch * 128:(tch + 1) * 128, None])
                    g = ep.tile([128, H], F32)
                    nc.gpsimd.indirect_dma_start(
                        out=g[:], out_offset=None, in_=word_emb[:],
                        in_offset=bass.IndirectOffsetOnAxis(ap=idx[:, :1], axis=0))
                    nc.vector.tensor_add(out=g[:], in0=g[:],
                                         in1=pptb[:, tch % (S // 128), :])
                    stats = ep.tile([128, 3, 6], F32)
                    for i in range(3):
                        nc.vector.bn_stats(out=stats[:, i, :],
                                           in_=g[:, i * 256:(i + 1) * 256])
                    mv = ep.tile([128, 2], F32)
                    nc.vector.bn_aggr(out=mv[:], in_=stats[:])
                    sd = ep.tile([128, 1], F32)
                    nc.scalar.activation(out=sd[:], in_=mv[:, 1:2], func=AF.Ln,
                                         bias=eps_e[:])
                    nc.scalar.activation(out=sd[:], in_=sd[:], func=AF.Exp,
                                         scale=-0.5)
                    xn = ep.tile([128, H], F32)
                    nc.vector.tensor_scalar(out=xn[:], in0=g[:], scalar1=mv[:, 0:1],
                                            scalar2=sd[:], op0=ALU.subtract,
                                            op1=ALU.mult)
                    nc.vector.tensor_mul(out=xn[:], in0=xn[:], in1=s_b[:])
                    nc.vector.tensor_add(out=xn[:], in0=xn[:], in1=b_b[:])
                    for fc in range(HC):
                        tp = ps_e.tile([128, 128], F32, space="PSUM")
                        nc.tensor.transpose(out=tp[:],
                                            in_=xn[:, fc * 128:(fc + 1) * 128],
                                            identity=identity[:])
                        nc.scalar.activation(out=xT[:, fc, tch * 128:(tch + 1) * 128],
                                             in_=tp[:], func=AF.Identity)

            for i in range(num_layers):
                _layer(tc, nc, i, i % L, xT, c)

            nc.sync.dma_start(
                out=xt_out[:].rearrange("(c p) t -> p c t", p=128), in_=xT[:])

    nc.compile()
    return nc


def _layernorm(tc, nc, lp, ps_st, ps_bc, xin, hout, s_col, b_col, c):
    """Feature-major LN: xin [128, HC, T] f32 -> hout [128, HC, T] bf16.

    Stats (incl. the Ln/Exp rstd ops) are computed for BOTH halves before any
    normalize work, so all table-needing ACT ops cluster (no gelu<->exp
    table ping-pong with neighboring FFN work)."""
    rows = []
    for n in range(NT):
        sl = slice(n * 512, (n + 1) * 512)
        xs_ps = ps_st.tile([1, 512], F32, space="PSUM", tag="xs", bufs=2)
        ss_ps = ps_st.tile([1, 512], F32, space="PSUM", tag="ss", bufs=2)
        for ch in range(HC):
            xb = lp.tile([128, 512], BF16, tag="xb", bufs=2)
            nc.scalar.activation(out=xb[:], in_=xin[:, ch, sl], func=AF.Identity)
            sq = lp.tile([128, 512], BF16, tag="sq", bufs=2)
            # gpsimd: the DVE FIFO is typically backed up with residual-add
            # work at LN entry; gpsimd is idle so the first stats matmul
            # isn't starved.
            nc.gpsimd.tensor_mul(out=sq[:], in0=xb[:], in1=xb[:])
            nc.tensor.matmul(out=xs_ps[:], lhsT=c.ones_col_bf[:], rhs=xb[:],
                             start=(ch == 0), stop=(ch == HC - 1))
            nc.tensor.matmul(out=ss_ps[:], lhsT=c.ones_col_bf[:], rhs=sq[:],
                             start=(ch == 0), stop=(ch == HC - 1))
        mu = lp.tile([1, 512], F32, tag="row", bufs=4)
        nc.scalar.activation(out=mu[:], in_=xs_ps[:], func=AF.Identity,
                             scale=1.0 / H)
        ex2 = lp.tile([1, 512], F32, tag="row", bufs=4)
        nc.scalar.activation(out=ex2[:], in_=ss_ps[:], func=AF.Identity,
                             scale=1.0 / H)
        var = lp.tile([1, 512], F32, tag="rowv", bufs=2)
        nc.vector.tensor_mul(out=var[:], in0=mu[:], in1=mu[:])
        nc.vector.tensor_sub(out=var[:], in0=ex2[:], in1=var[:])
        rows.append((mu, var))
    rstds = []
    for n, (mu, var) in enumerate(rows):
        nc.scalar.activation(out=var[:], in_=var[:], func=AF.Ln, bias=c.eps_l[:])
        nc.scalar.activation(out=var[:], in_=var[:], func=AF.Exp, scale=-0.5)
        # var now holds rstd = exp(-0.5*ln(var+eps)); ln/exp share an ACT table
        # set with softmax's exp, avoiding per-layer table reloads (sqrt does
        # not).
        rstds.append(var)
    for n in range(NT):
        sl = slice(n * 512, (n + 1) * 512)
        mu, rstd = rows[n][0], rstds[n]
        mu_b = lp.tile([128, 512], F32, tag="mu_b", bufs=2)
        rstd_b = lp.tile([128, 512], F32, tag="rstd_b", bufs=2)
        for row, bcast in ((mu, mu_b), (rstd, rstd_b)):
            bp = ps_bc.tile([128, 512], F32, space="PSUM", tag="bc", bufs=2)
            nc.tensor.matmul(out=bp[:], lhsT=c.ones_row[:],
                             rhs=row[:], start=True, stop=True)
            nc.scalar.activation(out=bcast[:], in_=bp[:], func=AF.Identity)
        for ch in range(HC):
            t1 = lp.tile([128, 512], F32, tag="t1", bufs=2)
            nc.vector.tensor_sub(out=t1[:], in0=xin[:, ch, sl], in1=mu_b[:])
            nc.vector.tensor_mul(out=t1[:], in0=t1[:], in1=rstd_b[:])
            nc.vector.tensor_scalar(out=hout[:, ch, sl], in0=t1[:],
                                    scalar1=s_col[:, ch:ch + 1],
                                    scalar2=b_col[:, ch:ch + 1],
                                    op0=ALU.mult, op1=ALU.add)


import contextlib


def _scope(nc, name):
    return nc.named_scope(name)


def _layer(tc, nc, idx, l, xT, c):
    with (
        tc.tile_pool(name=f"lp{idx}", bufs=2) as lp,
        tc.tile_pool(name=f"big{idx}", bufs=1) as bigp,
        tc.tile_pool(name=f"wp{idx}", bufs=8) as wp,
        tc.tile_pool(name=f"cst{idx}", bufs=1) as cst,
    ):
        ln_cols = cst.tile([128, 4 * HC], F32)
        nc.sync.dma_start(out=ln_cols[:],
                          in_=c.lnp[l].rearrange("k (c p) -> p (k c)", p=128))
        bq_cols = cst.tile([128, 4 * HC], F32)
        nc.sync.dma_start(out=bq_cols[:],
                          in_=c.bqkvo[l].rearrange("k (c p) -> p (k c)", p=128))
        b1_cols = cst.tile([128, FC], F32)
        nc.sync.dma_start(out=b1_cols[:],
                          in_=c.b1[l].rearrange("(c p) -> p c", p=128))
        b2_cols = cst.tile([128, HC], F32)
        nc.sync.dma_start(out=b2_cols[:],
                          in_=c.b2[l].rearrange("(c p) -> p c", p=128))
        bv_b = cst.tile([128, H], F32)
        nc.sync.dma_start(out=bv_b[:], in_=c.dram_bcast(c.bqkvo[l, 2], 128))

        hT = bigp.tile([128, HC, T], BF16, tag="hT", bufs=1)
        with (
            _scope(nc, f"l{idx:02d}.ln1"),
            tc.tile_pool(name=f"st{idx}a", bufs=1, space="PSUM") as ps_st,
            tc.tile_pool(name=f"bc{idx}a", bufs=2, space="PSUM") as ps_bc,
        ):
            _layernorm(tc, nc, lp, ps_st, ps_bc, xT, hT,
                       ln_cols[:, 0:HC], ln_cols[:, HC:2 * HC], c)

        # ---- V projection (its own PSUM pool; closes before attention) ----
        qT = bigp.tile([128, HC, T], BF16, tag="qT", bufs=1)
        kT = bigp.tile([128, HC, T], BF16, tag="kT", bufs=1)
        attnT = bigp.tile([128, HC, T], BF16, tag="attnT", bufs=1)
        wq_t, wk_t, wo_t = [], [], []
        for wmat, lst, tag in ((c.wq, wq_t, "wq"), (c.wk, wk_t, "wk"),
                               (c.wo, wo_t, "wo")):
            for ki in range(HC):
                wt = wp.tile([128, H], BF16, tag=tag, bufs=HC)
                nc.sync.dma_start(out=wt[:],
                                  in_=wmat[l, ki * 128:(ki + 1) * 128, :])
                lst.append(wt)
        with (
            _scope(nc, f"l{idx:02d}.qkv"),
            tc.tile_pool(name=f"vp{idx}", bufs=2, space="PSUM") as ps_v,
        ):
            vtiles = []
            for ki in range(HC):
                wt = wp.tile([128, H], BF16, tag="wv", bufs=HC)
                nc.sync.dma_start(out=wt[:], in_=c.wv[l, ki * 128:(ki + 1) * 128, :])
                vtiles.append(wt)
            v_sb = []
            for s in range(BPC):
                vt = bigp.tile([128, S // 128, NH, VH], BF16, tag="v", bufs=2)
                nc.vector.memset(vt[:, :, :, 64:65], 1.0)
                v_sb.append(vt)
            for tq in range(TC):
                ps = ps_v.tile([128, H], F32, space="PSUM", tag="vp")
                for ki in range(HC):
                    for n0, nn in ((0, 512), (512, 256)):
                        nc.tensor.matmul(
                            out=ps[:, n0:n0 + nn],
                            lhsT=hT[:, ki, tq * 128:(tq + 1) * 128],
                            rhs=vtiles[ki][:, n0:n0 + nn],
                            start=(ki == 0), stop=(ki == HC - 1))
                nc.vector.tensor_add(
                    out=v_sb[tq // 4][:, tq % 4, :, 0:64],
                    in0=ps[:].rearrange("p (h d) -> p h d", d=64),
                    in1=bv_b[:].rearrange("p (h d) -> p h d", d=64))

        # ---- per-sample pipeline: Q(s), K(s) -> attention(s) -> Wo(s) ----
        # One shared 8-bank PSUM budget: p(2) + sc(2x2) + au(2).  att(s0)
        # overlaps Q/K(s1) and Wo(s0); exp(ACT) overlaps projection matmuls.
        with (
            _scope(nc, f"l{idx:02d}.att"),
            tc.tile_pool(name=f"pp{idx}", bufs=2, space="PSUM") as ps_p,
            tc.tile_pool(name=f"sc{idx}", bufs=2, space="PSUM") as ps_sc,
            tc.tile_pool(name=f"au{idx}", bufs=2, space="PSUM") as ps_au,
        ):
            for s in range(BPC):
                nsl = slice(s * 512, (s + 1) * 512)
                for wtiles_, bofs, out_t in ((wq_t, 0, qT), (wk_t, HC, kT)):
                    for mo in range(HC):
                        ps = ps_p.tile([128, 512], F32, space="PSUM", tag="p")
                        for ki in range(HC):
                            nc.tensor.matmul(
                                out=ps[:],
                                lhsT=wtiles_[ki][:, mo * 128:(mo + 1) * 128],
                                rhs=hT[:, ki, nsl],
                                start=(ki == 0), stop=(ki == HC - 1))
                        nc.any.tensor_scalar_add(
                            out=out_t[:, mo, nsl], in0=ps[:],
                            scalar1=bq_cols[:, bofs + mo:bofs + mo + 1])
                # attention, head pairs packed into the PE array (row groups).
                # Denominator rows of 4 heads gather at 32-aligned partitions
                # {0,32,64,96} of one [128,512] tile (engine partition bases
                # must be 32-aligned) -> 3 batched reciprocals per sample.
                vt = v_sb[s]
                for h in range(NH):
                    hp = (h % 2) * 64
                    mo = h // 2
                    exs = []
                    for half in range(2):
                        sc = ps_sc.tile([128, 2, 512], F32, space="PSUM",
                                        tag="sc", bufs=2, name=f"sc_{half}")
                        for cki in range(2):
                            ck = half * 2 + cki
                            nc.tensor.matmul(
                                out=sc[:, cki, :],
                                lhsT=kT[hp:hp + 64, mo,
                                        s * 512 + ck * 128:
                                        s * 512 + (ck + 1) * 128],
                                rhs=qT[hp:hp + 64, mo, nsl],
                                start=True, stop=True)
                        ex = lp.tile([128, 2, 512], BF16, tag="exp", bufs=5,
                                     name=f"ex_{half}")
                        nc.scalar.activation(out=ex[:], in_=sc[:], func=AF.Exp,
                                             scale=0.125)
                        exs.append(ex)
                    au = ps_au.tile([VH, 512], F32, space="PSUM", tag="au")
                    for ck in range(4):
                        nc.tensor.matmul(out=au[:], lhsT=vt[:, ck, h, :],
                                         rhs=exs[ck // 2][:, ck % 2, :],
                                         start=(ck == 0), stop=(ck == 3))
                    # evict au fast (PSUM slot frees for the next head's
                    # matmuls), then 1/den = exp(-ln(den)) on ScalarE -- same
                    # ACT table set as softmax exp, keeps the DVE free.
                    den = lp.tile([1, 512], F32, tag="den", bufs=4)
                    nc.scalar.activation(out=den[:], in_=au[64:65, :],
                                         func=AF.Ln)
                    at = lp.tile([64, 512], BF16, tag="at", bufs=6)
                    nc.any.tensor_copy(out=at[:], in_=au[0:64, :])
                    rr = lp.tile([1, 512], BF16, tag="rra", bufs=4)
                    nc.scalar.activation(out=rr[:], in_=den[:], func=AF.Exp,
                                         scale=-1.0)
                    bc = lp.tile([64, 512], BF16, tag="bc", bufs=3)
                    nc.gpsimd.partition_broadcast(out_ap=bc[:], in_ap=rr[:])
                    nc.vector.tensor_mul(out=attnT[hp:hp + 64, mo, nsl],
                                         in0=at[:], in1=bc[:])
                # ---- attention output projection + residual (sample s) ----
                for mo in range(HC):
                    ps = ps_p.tile([128, 512], F32, space="PSUM", tag="p")
                    for ki in range(HC):
                        nc.tensor.matmul(
                            out=ps[:],
                            lhsT=wo_t[ki][:, mo * 128:(mo + 1) * 128],
                            rhs=attnT[:, ki, nsl],
                            start=(ki == 0), stop=(ki == HC - 1))
                    nc.vector.scalar_tensor_tensor(
                        out=xT[:, mo, nsl], in0=ps[:],
                        scalar=bq_cols[:, 3 * HC + mo:3 * HC + mo + 1],
                        in1=xT[:, mo, nsl], op0=ALU.add, op1=ALU.add)

        # ---- LN2 + FFN ----
        h2T = bigp.tile([128, HC, T], BF16, tag="hT", bufs=1)
        with (
            _scope(nc, f"l{idx:02d}.ln2"),
            tc.tile_pool(name=f"st{idx}b", bufs=1, space="PSUM") as ps_st,
            tc.tile_pool(name=f"bc{idx}b", bufs=2, space="PSUM") as ps_bc,
        ):
            _layernorm(tc, nc, lp, ps_st, ps_bc, xT, h2T,
                       ln_cols[:, 2 * HC:3 * HC], ln_cols[:, 3 * HC:4 * HC], c)

        with (
            _scope(nc, f"l{idx:02d}.ffn"),
            tc.tile_pool(name=f"f1{idx}", bufs=2, space="PSUM") as ps_f1,
            tc.tile_pool(name=f"f2{idx}", bufs=6, space="PSUM") as ps_f2,
        ):
            for n in range(NT):
                sl = slice(n * 512, (n + 1) * 512)
                f2s = []
                for _mo in range(HC):
                    f2t = ps_f2.tile([128, 512], F32, space="PSUM", tag="f2",
                                     bufs=6, name=f"f2_{idx}_{n}_{_mo}")
                    f2s.append(f2t)
                for k1b in range(FC // 4):
                    w1b = []
                    for ki in range(HC):
                        wt = wp.tile([128, 512], BF16, tag="w1b", bufs=7)
                        nc.sync.dma_start(
                            out=wt[:],
                            in_=c.w1[l, ki * 128:(ki + 1) * 128,
                                     k1b * 512:(k1b + 1) * 512])
                        w1b.append(wt)
                    for k1i in range(4):
                        k1 = k1b * 4 + k1i
                        f1 = ps_f1.tile([128, 512], F32, space="PSUM", tag="f1",
                                        bufs=2)
                        for ki in range(HC):
                            nc.tensor.matmul(
                                out=f1[:],
                                lhsT=w1b[ki][:, k1i * 128:(k1i + 1) * 128],
                                rhs=h2T[:, ki, sl],
                                start=(ki == 0), stop=(ki == HC - 1))
                        ffs = lp.tile([128, 512], BF16, tag="ffs", bufs=3)
                        nc.scalar.activation(out=ffs[:], in_=f1[:], func=AF.Gelu,
                                             bias=b1_cols[:, k1:k1 + 1])
                        w2t = wp.tile([128, H], BF16, tag="w2", bufs=4)
                        nc.sync.dma_start(out=w2t[:],
                                          in_=c.w2[l, k1 * 128:(k1 + 1) * 128, :])
                        for mo in range(HC):
                            nc.tensor.matmul(
                                out=f2s[mo][:],
                                lhsT=w2t[:, mo * 128:(mo + 1) * 128],
                                rhs=ffs[:],
                                start=(k1 == 0), stop=(k1 == FC - 1))
                for mo in range(HC):
                    nc.vector.scalar_tensor_tensor(
                        out=xT[:, mo, sl], in0=f2s[mo][:],
                        scalar=b2_cols[:, mo:mo + 1],
                        in1=xT[:, mo, sl], op0=ALU.add, op1=ALU.add)


_NC_CACHE = {}


def get_nc(num_layers=L):
    if num_layers not in _NC_CACHE:
        _NC_CACHE[num_layers] = build_nc(num_layers)
    return _NC_CACHE[num_layers]


def make_in_maps(inputs):
    bf = lambda a: np.ascontiguousarray(np.asarray(a, np.float32)).astype(
        ml_dtypes.bfloat16)
    f32 = lambda a: np.ascontiguousarray(np.asarray(a, np.float32))
    ids_all = np.asarray(inputs["input_ids"]).astype(np.int32)  # [16, 512]
    shared = {
        "word_emb": f32(inputs["word_emb"]),
        "ppt": f32(np.asarray(inputs["pos_emb"][:S], np.float32)
                   + np.asarray(inputs["tok_emb"][0], np.float32)),
        "ln_e": np.stack([f32(inputs["ln_e_s"]), f32(inputs["ln_e_b"])]),
        "lnp": np.stack([f32(inputs["ln1_s"]), f32(inputs["ln1_b"]),
                         f32(inputs["ln2_s"]), f32(inputs["ln2_b"])], axis=1),
        "wq": bf(inputs["Wq"]), "wk": bf(inputs["Wk"]),
        "wv": bf(inputs["Wv"]), "wo": bf(inputs["Wo"]),
        "w1": bf(inputs["W1"]), "w2": bf(inputs["W2"]),
        "bqkvo": np.stack([f32(inputs["bq"]), f32(inputs["bk"]),
                           f32(inputs["bv"]), f32(inputs["bo"])], axis=1),
        "b1": f32(inputs["b1"]), "b2": f32(inputs["b2"]),
    }
    return [
        {"ids": ids_all[c * BPC:(c + 1) * BPC].reshape(-1), **shared}
        for c in range(NCORES)
    ]


def assemble(results):
    outs = []
    for c in range(NCORES):
        xt = results[c]["xt_out"]  # [768, 1024]
        outs.append(np.ascontiguousarray(np.asarray(xt, np.float32).T)
                    .reshape(BPC, S, H))
    return np.concatenate(outs, axis=0)


def kernel(**inputs) -> np.ndarray:
    nc = get_nc()
    in_maps = make_in_maps(inputs)
    res = run_bass_kernel_spmd(nc, in_maps, list(range(NCORES)))
    return assemble(res.results)


if __name__ == "__main__":
    nl = int(sys.argv[1]) if len(sys.argv) > 1 else 1
    nc = build_nc(nl)
    print("build ok", nl)



# revision 16
# speedup vs baseline: 1.2962x; 1.2962x over previous
"""BERT encoder (B=16, S=512, H=768, L=12, F=3072, NH=12) on 8 trn2 NeuronCores.

Sharding: pure data-parallel over batch -- each core processes 2 samples
(1024 tokens).  Weights replicated (bf16 host-side); activations feature-major
on-chip: xT[f, t], so every linear is matmul(out=yT, lhsT=W, rhs=xT).

v2: the whole layer is a per-sample software pipeline.  Phases per sample
(A=LN1, B=V/Q/K, C=attention, D=Wo+residual, E=LN2, F=FFN) are emitted in the
order A0 B0 A1 C0 B1 D0 E0 C1 F0 A0' D1 E1 F1 so the Tile list-scheduler can
fill every cross-engine latency (softmax exp, LN row math, reciprocal chain)
with independent matmuls from the other sample -- the PE never sees a >3.4us
gap, which also keeps the HAM clock-gate at 2.4GHz.  A0' is the NEXT layer's
LN1(s0), emitted before F1 so its Ln op (which triggers the exp/ln ACT-table
reload after FFN's gelu) and its DVE chain hide under F-phase matmuls.

PSUM budget (8 banks): proj/stats ring 3 x [128,512], ffn ring 2, score ring
2, attention-AV 1.  LN stats for x and x^2 pack into rows {0,32} of ONE bank
via auto tile_position.  LN runs on a bf16 copy of x (DVE 2x mode); mu/rstd
row math stays f32; per-token rows broadcast with idle-GpSimd
partition_broadcast instead of PE matmuls.  Softmax denominator rides the
appended ones-column of V (row 64 of the AV psum); 1/den = exp(-ln(den)) on
ScalarE straight out of PSUM (same ACT table set as softmax exp).
Residual stream fp32; matmul operands bf16.
"""

import sys

for _p in ("/opt/trn_rl_repo",):
    if _p not in sys.path:
        sys.path.insert(0, _p)

import numpy as np
import ml_dtypes

import concourse.bass as bass
import concourse.tile as tile
from concourse import bacc, mybir
from concourse.bass_utils import run_bass_kernel_spmd
from concourse.masks import make_identity

AF = mybir.ActivationFunctionType
ALU = mybir.AluOpType
F32 = mybir.dt.float32
BF16 = mybir.dt.bfloat16
I32 = mybir.dt.int32

B, S, H, L, FF, V, NH = 16, 512, 768, 12, 3072, 30522, 12
HD = H // NH  # 64
NCORES = 8
BPC = B // NCORES  # samples per core = 2
T = BPC * S  # tokens per core = 1024
HC = H // 128  # feature chunks = 6
FC = FF // 128  # ffn chunks = 24
TC = T // 128
EPS_EMB, EPS_LN = 1e-12, 1e-5
VH = 65  # per-head v columns: 64 v + 1 ones (denominator trick)


class Ctx:
    pass


def _scope(nc, name):
    return nc.named_scope(name)


def build_nc(num_layers=L):
    nc = bacc.Bacc("TRN2", target_bir_lowering=False, debug=False,
                   num_devices=NCORES)

    ids = nc.declare_dram_parameter("ids", [T], I32, isOutput=False)
    word_emb = nc.declare_dram_parameter("word_emb", [V, H], F32, isOutput=False)
    ppt = nc.declare_dram_parameter("ppt", [S, H], F32, isOutput=False)
    ln_e = nc.declare_dram_parameter("ln_e", [2, H], F32, isOutput=False)
    c = Ctx()
    c.lnp = nc.declare_dram_parameter("lnp", [L, 4, H], F32, isOutput=False)
    c.wq = nc.declare_dram_parameter("wq", [L, H, H], BF16, isOutput=False)
    c.wk = nc.declare_dram_parameter("wk", [L, H, H], BF16, isOutput=False)
    c.wv = nc.declare_dram_parameter("wv", [L, H, H], BF16, isOutput=False)
    c.wo = nc.declare_dram_parameter("wo", [L, H, H], BF16, isOutput=False)
    c.w1 = nc.declare_dram_parameter("w1", [L, H, FF], BF16, isOutput=False)
    c.w2 = nc.declare_dram_parameter("w2", [L, FF, H], BF16, isOutput=False)
    c.bqkvo = nc.declare_dram_parameter("bqkvo", [L, 4, H], F32, isOutput=False)
    c.b1 = nc.declare_dram_parameter("b1", [L, FF], F32, isOutput=False)
    c.b2 = nc.declare_dram_parameter("b2", [L, H], F32, isOutput=False)
    xt_out = nc.declare_dram_parameter("xt_out", [H, T], F32, isOutput=True)

    def dram_bcast(ap_1d, parts):
        a = ap_1d
        return bass.AP(tensor=a.tensor, offset=a.offset, ap=[[0, parts], *a.ap])

    c.dram_bcast = dram_bcast

    with tile.TileContext(nc) as tc:
        with (
            tc.tile_pool(name="persist", bufs=1) as pp,
            tc.tile_pool(name="xpool", bufs=1) as xp,
        ):
            identity = pp.tile([128, 128], F32)
            make_identity(nc, identity[:])
            c.ones_col_bf = pp.tile([128, 1], BF16)
            nc.vector.memset(c.ones_col_bf[:], 1.0)
            eps_e = pp.tile([128, 1], F32)
            nc.vector.memset(eps_e[:], EPS_EMB)
            c.eps_l = pp.tile([1, 1], F32)
            nc.vector.memset(c.eps_l[:], EPS_LN)

            xT = xp.tile([128, HC, T], F32)  # residual stream, feature-major

            # ---------------- embedding (one-shot, own pools) ----------------
            with (
                tc.tile_pool(name="emb", bufs=2) as ep,
                tc.tile_pool(name="embc", bufs=1) as ec,
                tc.tile_pool(name="embps", bufs=2, space="PSUM") as ps_e,
            ):
                s_b = ec.tile([128, H], F32)
                nc.sync.dma_start(out=s_b[:], in_=dram_bcast(ln_e[0], 128))
                b_b = ec.tile([128, H], F32)
                nc.sync.dma_start(out=b_b[:], in_=dram_bcast(ln_e[1], 128))
                pptb = ec.tile([128, S // 128, H], F32)
                nc.sync.dma_start(
                    out=pptb[:], in_=ppt[:].rearrange("(c p) h -> p c h", p=128))
                for tch in range(TC):
                    idx = ep.tile([128, 1], I32)
                    nc.sync.dma_start(out=idx[:],
                                      in_=ids[tch * 128:(tch + 1) * 128, None])
                    g = ep.tile([128, H], F32)
                    nc.gpsimd.indirect_dma_start(
                        out=g[:], out_offset=None, in_=word_emb[:],
                        in_offset=bass.IndirectOffsetOnAxis(ap=idx[:, :1], axis=0))
                    nc.vector.tensor_add(out=g[:], in0=g[:],
                                         in1=pptb[:, tch % (S // 128), :])
                    stats = ep.tile([128, 3, 6], F32)
                    for i in range(3):
                        nc.vector.bn_stats(out=stats[:, i, :],
                                           in_=g[:, i * 256:(i + 1) * 256])
                    mv = ep.tile([128, 2], F32)
                    nc.vector.bn_aggr(out=mv[:], in_=stats[:])
                    sd = ep.tile([128, 1], F32)
                    nc.scalar.activation(out=sd[:], in_=mv[:, 1:2], func=AF.Ln,
                                         bias=eps_e[:])
                    nc.scalar.activation(out=sd[:], in_=sd[:], func=AF.Exp,
                                         scale=-0.5)
                    xn = ep.tile([128, H], F32)
                    nc.vector.tensor_scalar(out=xn[:], in0=g[:], scalar1=mv[:, 0:1],
                                            scalar2=sd[:], op0=ALU.subtract,
                                            op1=ALU.mult)
                    nc.vector.tensor_mul(out=xn[:], in0=xn[:], in1=s_b[:])
                    nc.vector.tensor_add(out=xn[:], in0=xn[:], in1=b_b[:])
                    for fc in range(HC):
                        tp = ps_e.tile([128, 128], F32, space="PSUM")
                        nc.tensor.transpose(out=tp[:],
                                            in_=xn[:, fc * 128:(fc + 1) * 128],
                                            identity=identity[:])
                        nc.scalar.activation(out=xT[:, fc, tch * 128:(tch + 1) * 128],
                                             in_=tp[:], func=AF.Identity)

            # ---------------- hoisted layer pools ----------------
            P = Ctx()
            with (
                tc.tile_pool(name="wts", bufs=1) as wp,
                tc.tile_pool(name="act", bufs=1) as ap_,
                tc.tile_pool(name="rows", bufs=1) as rp,
                tc.tile_pool(name="lps", bufs=1, space="PSUM") as ps,
            ):
                P.wp, P.ap, P.rp, P.ps = wp, ap_, rp, ps
                ln0 = rp.tile([128, 4 * HC], F32, tag="lncols", bufs=3,
                              name="lncols_pre")
                nc.sync.dma_start(out=ln0[:],
                                  in_=c.lnp[0].rearrange("k (c p) -> p (k c)",
                                                         p=128))
                # A-phase state for the software pipeline: hT(s0) for layer 0
                P.a0_hT = _ln(tc, nc, P, c, xT, 0, 0, "pre.a0", ln0)
                for i in range(num_layers):
                    _layer(tc, nc, i, i % L, xT, c, P,
                           next_l=((i + 1) % L) if (i + 1 < num_layers) else None)

            nc.sync.dma_start(
                out=xt_out[:].rearrange("(c p) t -> p c t", p=128), in_=xT[:])

    nc.compile()
    return nc


def _ln(tc, nc, P, c, xT, s, lncols_base, scope, lncols):
    """LN over tokens [s*512,(s+1)*512): stats via packed PE rows, bf16
    normalize.  Returns the normalized hT tile [128, HC, 512] bf16."""
    ap_, rp, ps = P.ap, P.rp, P.ps
    sl = slice(s * 512, (s + 1) * 512)
    with _scope(nc, scope):
        st = ps.tile([128, 512], F32, space="PSUM", tag="proj", bufs=3,
                     name=f"st_{scope}")
        xbs = []
        for ki in range(HC):
            xb = ap_.tile([128, 512], BF16, tag="xb", bufs=8,
                          name=f"xb_{scope}_{ki}")
            nc.vector.tensor_copy(out=xb[:], in_=xT[:, ki, sl])
            sq = ap_.tile([128, 512], BF16, tag="sq", bufs=1,
                          name=f"sq_{scope}_{ki}")
            nc.gpsimd.tensor_mul(out=sq[:], in0=xb[:], in1=xb[:])
            nc.tensor.matmul(out=st[0:1, :], lhsT=c.ones_col_bf[:], rhs=xb[:],
                             start=(ki == 0), stop=(ki == HC - 1))
            nc.tensor.matmul(out=st[32:33, :], lhsT=c.ones_col_bf[:], rhs=sq[:],
                             start=(ki == 0), stop=(ki == HC - 1))
            xbs.append(xb)
        mu = rp.tile([1, 512], F32, tag="mu", bufs=1, name=f"mu_{scope}")
        nc.vector.tensor_scalar_mul(out=mu[:], in0=st[0:1, :], scalar1=1.0 / H)
        musq = rp.tile([1, 512], F32, tag="musq", bufs=1, name=f"musq_{scope}")
        nc.scalar.activation(out=musq[:], in_=st[0:1, :], func=AF.Square,
                             scale=1.0 / H)
        var = rp.tile([1, 512], F32, tag="var", bufs=1, name=f"var_{scope}")
        nc.vector.scalar_tensor_tensor(out=var[:], in0=st[32:33, :],
                                       scalar=1.0 / H, in1=musq[:],
                                       op0=ALU.mult, op1=ALU.subtract)
        nc.scalar.activation(out=var[:], in_=var[:], func=AF.Ln, bias=c.eps_l[:])
        nc.scalar.activation(out=var[:], in_=var[:], func=AF.Exp, scale=-0.5)
        # var now holds rstd (f32)
        m2 = rp.tile([1, 512], BF16, tag="m2", bufs=1, name=f"m2_{scope}")
        nc.vector.tensor_mul(out=m2[:], in0=mu[:], in1=var[:])
        rstd_bf = rp.tile([1, 512], BF16, tag="rstdb", bufs=1,
                          name=f"rstdb_{scope}")
        nc.vector.tensor_copy(out=rstd_bf[:], in_=var[:])
        rstd_b = ap_.tile([128, 512], BF16, tag="rstd_b", bufs=2,
                          name=f"rstdB_{scope}")
        nc.gpsimd.partition_broadcast(out_ap=rstd_b[:], in_ap=rstd_bf[:])
        m2_b = ap_.tile([128, 512], BF16, tag="m2_b", bufs=2,
                        name=f"m2B_{scope}")
        nc.gpsimd.partition_broadcast(out_ap=m2_b[:], in_ap=m2[:])
        hT = ap_.tile([128, HC, 512], BF16, tag="h", bufs=3,
                      name=f"hT_{scope}")
        lc = lncols
        for ki in range(HC):
            t1 = ap_.tile([128, 512], BF16, tag="t1", bufs=1,
                          name=f"t1_{scope}_{ki}")
            nc.vector.tensor_mul(out=t1[:], in0=xbs[ki][:], in1=rstd_b[:])
            t2 = ap_.tile([128, 512], BF16, tag="t2", bufs=1,
                          name=f"t2_{scope}_{ki}")
            nc.vector.tensor_sub(out=t2[:], in0=t1[:], in1=m2_b[:])
            nc.vector.tensor_scalar(
                out=hT[:, ki, :], in0=t2[:],
                scalar1=lc[:, lncols_base + ki:lncols_base + ki + 1],
                scalar2=lc[:, lncols_base + HC + ki:lncols_base + HC + ki + 1],
                op0=ALU.mult, op1=ALU.add)
        return hT


def _layer(tc, nc, idx, l, xT, c, P, next_l=None):
    wp, ap_, rp, ps = P.wp, P.ap, P.rp, P.ps

    # ---- per-layer constants ----
    ln_cols = rp.tile([128, 4 * HC], F32, tag="lncols", bufs=3,
                      name=f"lncols_{idx}")
    nc.sync.dma_start(out=ln_cols[:],
                      in_=c.lnp[l].rearrange("k (c p) -> p (k c)", p=128))
    bq_cols = rp.tile([128, 4 * HC], F32, tag="bqcols", bufs=2,
                      name=f"bqcols_{idx}")
    nc.sync.dma_start(out=bq_cols[:],
                      in_=c.bqkvo[l].rearrange("k (c p) -> p (k c)", p=128))
    b1_cols = rp.tile([128, FC], F32, tag="b1cols", bufs=2, name=f"b1c_{idx}")
    nc.sync.dma_start(out=b1_cols[:],
                      in_=c.b1[l].rearrange("(c p) -> p c", p=128))
    b2_cols = rp.tile([128, HC], F32, tag="b2cols", bufs=2, name=f"b2c_{idx}")
    nc.sync.dma_start(out=b2_cols[:],
                      in_=c.b2[l].rearrange("(c p) -> p c", p=128))
    bv_b = rp.tile([128, H], BF16, tag="bvb", bufs=2, name=f"bvb_{idx}")
    nc.gpsimd.dma_start(out=bv_b[:], in_=c.dram_bcast(c.bqkvo[l, 2], 128))

    # ---- weight tiles (consumed by both samples; ring carries prefetch) ----
    wq_t, wk_t, wv_t, wo_t = [], [], [], []
    for wmat, lst, tag in ((c.wq, wq_t, "wq"), (c.wk, wk_t, "wk"),
                           (c.wv, wv_t, "wv"), (c.wo, wo_t, "wo")):
        for ki in range(HC):
            wt = wp.tile([128, H], BF16, tag=tag, bufs=HC,
                         name=f"{tag}_{idx}_{ki}")
            nc.sync.dma_start(out=wt[:], in_=wmat[l, ki * 128:(ki + 1) * 128, :])
            lst.append(wt)

    qT, kT, attnT, vt = [None, None], [None, None], [None, None], [None, None]

    def phase_b(s, hT):
        """V, Q, K projections for sample s."""
        with _scope(nc, f"l{idx:02d}.b{s}"):
            v = ap_.tile([128, S // 128, NH, VH], BF16, tag="vt", bufs=2,
                         name=f"vt_{idx}_{s}")
            nc.gpsimd.memset(v[:, :, :, 64:65], 1.0)
            for tq in range(S // 128):
                psA = ps.tile([128, 512], F32, space="PSUM", tag="proj",
                              bufs=3, name=f"vA_{idx}_{s}_{tq}")
                psB = ps.tile([128, 512], F32, space="PSUM", tag="proj",
                              bufs=3, name=f"vB_{idx}_{s}_{tq}")
                for ki in range(HC):
                    nc.tensor.matmul(
                        out=psA[:],
                        lhsT=hT[:, ki, tq * 128:(tq + 1) * 128],
                        rhs=wv_t[ki][:, 0:512],
                        start=(ki == 0), stop=(ki == HC - 1))
                    nc.tensor.matmul(
                        out=psB[:, 0:256],
                        lhsT=hT[:, ki, tq * 128:(tq + 1) * 128],
                        rhs=wv_t[ki][:, 512:768],
                        start=(ki == 0), stop=(ki == HC - 1))
                nc.vector.tensor_add(
                    out=v[:, tq, 0:8, 0:64],
                    in0=psA[:].rearrange("p (h d) -> p h d", d=64),
                    in1=bv_b[:, 0:512].rearrange("p (h d) -> p h d", d=64))
                nc.vector.tensor_add(
                    out=v[:, tq, 8:12, 0:64],
                    in0=psB[:, 0:256].rearrange("p (h d) -> p h d", d=64),
                    in1=bv_b[:, 512:768].rearrange("p (h d) -> p h d", d=64))
            vt[s] = v
            q = ap_.tile([128, HC, 512], BF16, tag="qk", bufs=4,
                         name=f"qT_{idx}_{s}")
            k = ap_.tile([128, HC, 512], BF16, tag="qk", bufs=4,
                         name=f"kT_{idx}_{s}")
            for wtiles_, bofs, out_t in ((wq_t, 0, q), (wk_t, HC, k)):
                for mo in range(HC):
                    p_ = ps.tile([128, 512], F32, space="PSUM", tag="proj",
                                 bufs=3, name=f"qk_{idx}_{s}_{bofs}_{mo}")
                    for ki in range(HC):
                        nc.tensor.matmul(
                            out=p_[:],
                            lhsT=wtiles_[ki][:, mo * 128:(mo + 1) * 128],
                            rhs=hT[:, ki, :],
                            start=(ki == 0), stop=(ki == HC - 1))
                    nc.vector.tensor_scalar_add(
                        out=out_t[:, mo, :], in0=p_[:],
                        scalar1=bq_cols[:, bofs + mo:bofs + mo + 1])
            qT[s], kT[s] = q, k

    def phase_c(s):
        """Attention for sample s."""
        with _scope(nc, f"l{idx:02d}.att{s}"):
            at_s = ap_.tile([128, HC, 512], BF16, tag="attnT", bufs=2,
                            name=f"attnT_{idx}_{s}")
            q, k, v = qT[s], kT[s], vt[s]
            for h in range(NH):
                hp = (h % 2) * 64
                mo = h // 2
                exs = []
                for ck in range(4):
                    sc = ps.tile([128, 512], F32, space="PSUM", tag="sc",
                                 bufs=2, name=f"sc_{idx}_{s}_{h}_{ck}")
                    nc.tensor.matmul(
                        out=sc[:],
                        lhsT=k[hp:hp + 64, mo, ck * 128:(ck + 1) * 128],
                        rhs=q[hp:hp + 64, mo, :],
                        start=True, stop=True)
                    ex = ap_.tile([128, 512], BF16, tag="ex", bufs=4,
                                  name=f"ex_{idx}_{s}_{h}_{ck}")
                    nc.scalar.activation(out=ex[:], in_=sc[:], func=AF.Exp,
                                         scale=0.125)
                    exs.append(ex)
                au = ps.tile([128, 512], F32, space="PSUM", tag="au", bufs=1,
                             name=f"au_{idx}_{s}_{h}")
                for ck in range(4):
                    nc.tensor.matmul(out=au[0:VH, :], lhsT=v[:, ck, h, :],
                                     rhs=exs[ck][:], start=(ck == 0),
                                     stop=(ck == 3))
                den = rp.tile([1, 512], F32, tag="den", bufs=1,
                              name=f"den_{idx}_{s}_{h}")
                nc.scalar.activation(out=den[:], in_=au[64:65, :], func=AF.Ln)
                rr = rp.tile([1, 512], BF16, tag="rr", bufs=2,
                             name=f"rr_{idx}_{s}_{h}")
                nc.scalar.activation(out=rr[:], in_=den[:], func=AF.Exp,
                                     scale=-1.0)
                at = ap_.tile([64, 512], BF16, tag="at", bufs=1,
                              name=f"at_{idx}_{s}_{h}")
                nc.vector.tensor_copy(out=at[:], in_=au[0:64, :])
                bc = ap_.tile([64, 512], BF16, tag="bc", bufs=1,
                              name=f"bc_{idx}_{s}_{h}")
                nc.gpsimd.partition_broadcast(out_ap=bc[:], in_ap=rr[:])
                nc.vector.tensor_mul(out=at_s[hp:hp + 64, mo, :],
                                     in0=at[:], in1=bc[:])
            attnT[s] = at_s

    def phase_d(s):
        """Wo + residual for sample s."""
        sl = slice(s * 512, (s + 1) * 512)
        with _scope(nc, f"l{idx:02d}.wo{s}"):
            for mo in range(HC):
                p_ = ps.tile([128, 512], F32, space="PSUM", tag="proj",
                             bufs=3, name=f"wo_{idx}_{s}_{mo}")
                for ki in range(HC):
                    nc.tensor.matmul(
                        out=p_[:],
                        lhsT=wo_t[ki][:, mo * 128:(mo + 1) * 128],
                        rhs=attnT[s][:, ki, :],
                        start=(ki == 0), stop=(ki == HC - 1))
                nc.vector.scalar_tensor_tensor(
                    out=xT[:, mo, sl], in0=p_[:],
                    scalar=bq_cols[:, 3 * HC + mo:3 * HC + mo + 1],
                    in1=xT[:, mo, sl], op0=ALU.add, op1=ALU.add)

    def phase_f(s, h2):
        """FFN + residual for sample s."""
        sl = slice(s * 512, (s + 1) * 512)
        with _scope(nc, f"l{idx:02d}.ffn{s}"):
            ffs = []
            for k1 in range(FC):
                w1t = wp.tile([128, HC, 128], BF16, tag="w1", bufs=4,
                              name=f"w1_{idx}_{s}_{k1}")
                nc.sync.dma_start(
                    out=w1t[:],
                    in_=c.w1[l][:, k1 * 128:(k1 + 1) * 128]
                    .rearrange("(c p) f -> p c f", p=128))
                f1 = ps.tile([128, 512], F32, space="PSUM", tag="ff", bufs=2,
                             name=f"f1_{idx}_{s}_{k1}")
                for ki in range(HC):
                    nc.tensor.matmul(out=f1[:], lhsT=w1t[:, ki, :],
                                     rhs=h2[:, ki, :],
                                     start=(ki == 0), stop=(ki == HC - 1))
                ft = ap_.tile([128, 512], BF16, tag="ffs", bufs=25,
                              name=f"ffs_{idx}_{s}_{k1}")
                nc.scalar.activation(out=ft[:], in_=f1[:], func=AF.Gelu,
                                     bias=b1_cols[:, k1:k1 + 1])
                ffs.append(ft)
            for mo in range(HC):
                w2t = wp.tile([128, FC, 128], BF16, tag="w2", bufs=2,
                              name=f"w2_{idx}_{s}_{mo}")
                nc.sync.dma_start(
                    out=w2t[:],
                    in_=c.w2[l][:, mo * 128:(mo + 1) * 128]
                    .rearrange("(c p) f -> p c f", p=128))
                f2 = ps.tile([128, 512], F32, space="PSUM", tag="ff", bufs=2,
                             name=f"f2_{idx}_{s}_{mo}")
                for k1 in range(FC):
                    nc.tensor.matmul(out=f2[:], lhsT=w2t[:, k1, :],
                                     rhs=ffs[k1][:],
                                     start=(k1 == 0), stop=(k1 == FC - 1))
                nc.vector.scalar_tensor_tensor(
                    out=xT[:, mo, sl], in0=f2[:],
                    scalar=b2_cols[:, mo:mo + 1],
                    in1=xT[:, mo, sl], op0=ALU.add, op1=ALU.add)

    # ---- pipeline emission ----
    # A0 was emitted by the previous layer (or the pre-loop call).
    # Order: B0 A1 C0 B1 D0 E0 C1 D1 E1 A0' F0 F1 -- every serial
    # DVE/ScalarE chain is emitted ahead of independent PE work that can
    # hide it; A0' (next layer's LN1 s0) sits before F0/F1 so its table
    # reload + chain hide under FFN matmuls.
    phase_b(0, P.a0_hT)
    h1_1 = _ln(tc, nc, P, c, xT, 1, 0, f"l{idx:02d}.a1", ln_cols)
    phase_c(0)
    phase_b(1, h1_1)
    phase_d(0)
    h2_0 = _ln(tc, nc, P, c, xT, 0, 2 * HC, f"l{idx:02d}.e0", ln_cols)
    phase_c(1)
    phase_d(1)
    h2_1 = _ln(tc, nc, P, c, xT, 1, 2 * HC, f"l{idx:02d}.e1", ln_cols)
    phase_f(0, h2_0)
    if next_l is not None:
        # Next layer's LN1(s0), emitted between F0 and F1: reads xT(s0)
        # after F0's residual; its ACT-table reload + DVE chain hide under
        # F1's matmuls.
        nl_cols = rp.tile([128, 4 * HC], F32, tag="lncols", bufs=3,
                          name=f"lncols_n{idx}")
        nc.sync.dma_start(out=nl_cols[:],
                          in_=c.lnp[next_l].rearrange("k (c p) -> p (k c)",
                                                      p=128))
        P.a0_hT = _ln(tc, nc, P, c, xT, 0, 0, f"l{idx:02d}.a0n", nl_cols)
    phase_f(1, h2_1)


_NC_CACHE = {}


def get_nc(num_layers=L):
    if num_layers not in _NC_CACHE:
        _NC_CACHE[num_layers] = build_nc(num_layers)
    return _NC_CACHE[num_layers]


def make_in_maps(inputs):
    bf = lambda a: np.ascontiguousarray(np.asarray(a, np.float32)).astype(
        ml_dtypes.bfloat16)
    f32 = lambda a: np.ascontiguousarray(np.asarray(a, np.float32))
    ids_all = np.asarray(inputs["input_ids"]).astype(np.int32)  # [16, 512]
    shared = {
        "word_emb": f32(inputs["word_emb"]),
        "ppt": f32(np.asarray(inputs["pos_emb"][:S], np.float32)
                   + np.asarray(inputs["tok_emb"][0], np.float32)),
        "ln_e": np.stack([f32(inputs["ln_e_s"]), f32(inputs["ln_e_b"])]),
        "lnp": np.stack([f32(inputs["ln1_s"]), f32(inputs["ln1_b"]),
                         f32(inputs["ln2_s"]), f32(inputs["ln2_b"])], axis=1),
        "wq": bf(inputs["Wq"]), "wk": bf(inputs["Wk"]),
        "wv": bf(inputs["Wv"]), "wo": bf(inputs["Wo"]),
        "w1": bf(inputs["W1"]), "w2": bf(inputs["W2"]),
        "bqkvo": np.stack([f32(inputs["bq"]), f32(inputs["bk"]),
                           f32(inputs["bv"]), f32(inputs["bo"])], axis=1),
        "b1": f32(inputs["b1"]), "b2": f32(inputs["b2"]),
    }
    return [
        {"ids": ids_all[c * BPC:(c + 1) * BPC].reshape(-1), **shared}
        for c in range(NCORES)
    ]


def assemble(results):
    outs = []
    for c in range(NCORES):
        xt = results[c]["xt_out"]  # [768, 1024]
        outs.append(np.ascontiguousarray(np.asarray(xt, np.float32).T)
                    .reshape(BPC, S, H))
    return np.concatenate(outs, axis=0)


def kernel(**inputs) -> np.ndarray:
    nc = get_nc()
    in_maps = make_in_maps(inputs)
    res = run_bass_kernel_spmd(nc, in_maps, list(range(NCORES)))
    return assemble(res.results)


if __name__ == "__main__":
    nl = int(sys.argv[1]) if len(sys.argv) > 1 else 1
    nc = build_nc(nl)
    print("build ok", nl)
